# revision 1
# baseline (speedup 1.0000x reference)
"""Trainium2 Bass kernel for BasicS2Conv (8-core SPMD).

Fast path (icosahedral inputs): fp32r kernel exploiting the separable
index structure -- kidx[k,a,r]=class(k,r) (sizes 1,5,5,1,1), aidx a
per-r permutation -- with T = sum of the 12 vertex slices and the
level-2 ring folded into T via host-modified weights, reducing the
contraction to 368,640 PE columns/iter (2.6x less than direct
binning). fp32r keeps matmuls self-loading (bf16 pays an unmodeled
~300ns/matmul LDWEIGHTS penalty on TRN2 hardware). 256 columns
(b-pair x PSH); all 12 r's in 6 PSUM banks (half-bank groups opened
by zero-stationary 512-col matmuls); x streamed per (b-pair, k-slice)
through a 5-slot pool; T chain + 7 adjacency accumulators split
across Pool/DVE with Act-copy inits; 5 r's' adjacency terms run as
direct matmuls (keeps the tile pools deadlock-free); per-bank drains
on Act.

Generic fallback (arbitrary kidx/aidx): the original binned fp32r
kernel.
"""

import sys

if "/opt/trn_rl_repo" not in sys.path:
    sys.path.insert(0, "/opt/trn_rl_repo")

from contextlib import ExitStack

import numpy as np

B, C, D, K, A, R, P = 4, 128, 128, 13, 12, 12, 1024
NCORES = 8
PSH = P // NCORES
KK = 5
NOFF = 8  # offloaded bins per r (SBUF-bounded)
MINMULT = 3  # only offload bins with at least this many sources

_cache: dict = {}


def _plan(kidx, aidx):
    """Per r: split the K*A (k,a) pairs into direct matmuls and
    offloaded bins (same (kk,aa), x slices pre-summed on DVE)."""
    plan = []
    for r in range(R):
        bins: dict = {}
        for k in range(K):
            for a in range(A):
                bins.setdefault(
                    (int(kidx[k, a, r]), int(aidx[k, a, r])), []
                ).append((k, a))
        cand = sorted(bins.items(), key=lambda kv: len(kv[1]), reverse=True)
        offload = [
            (kkaa, srcs) for kkaa, srcs in cand[:NOFF] if len(srcs) >= MINMULT
        ]
        off_set = set()
        for _, srcs in offload:
            off_set.update(srcs)
        direct = [
            (k, a, int(kidx[k, a, r]), int(aidx[k, a, r]))
            for k in range(K)
            for a in range(A)
            if (k, a) not in off_set
        ]
        plan.append((direct, offload))
    return plan


def _build(kidx: np.ndarray, aidx: np.ndarray, iters: int = 1):
    import concourse.bass as bass  # noqa: F401
    import concourse.tile as tile
    from concourse import bacc, mybir

    f32 = mybir.dt.float32
    f32r = mybir.dt.float32r

    plan = _plan(kidx, aidx)

    nc = bacc.Bacc(
        "TRN2", target_bir_lowering=False, debug=False, num_devices=NCORES
    )
    x_dram = nc.dram_tensor("x", [C, K, A, B, PSH], f32r, kind="ExternalInput").ap()
    w_dram = nc.dram_tensor("w", [C, KK, A, D], f32r, kind="ExternalInput").ap()
    out_dram = nc.dram_tensor("out", [B, D, PSH, R], f32, kind="ExternalOutput").ap()

    with tile.TileContext(nc) as tc, ExitStack() as ctx:
        wpool = ctx.enter_context(tc.tile_pool(name="wpool", bufs=1))
        xpool = ctx.enter_context(tc.tile_pool(name="xpool", bufs=2))
        opool = ctx.enter_context(tc.tile_pool(name="opool", bufs=1))
        bpool = ctx.enter_context(tc.tile_pool(name="bpool", bufs=1))
        ppool = ctx.enter_context(tc.tile_pool(name="ppool", bufs=1, space="PSUM"))

        w_t = wpool.tile([C, KK, A, D], f32r)
        nc.sync.dma_start(w_t[:], w_dram[:])

        psum = ppool.tile([D, 6, B, PSH], f32)  # 6 banks of 512 fp32
        bins_t = bpool.tile([C, 6, NOFF, B, PSH], f32r)
        out_s = opool.tile([D, B, PSH, R], f32)

        for _ in range(iters):
            for h in range(2):
                rs = [6 * h + j for j in range(6)]
                direct_by_k = [[[] for _ in range(K)] for _ in range(6)]
                for j, r in enumerate(rs):
                    for (k, a, kk, aa) in plan[r][0]:
                        direct_by_k[j][k].append((a, kk, aa))
                first_direct = []
                last_direct = []
                for j in range(6):
                    seq = [
                        (k, i)
                        for k in range(K)
                        for i in range(len(direct_by_k[j][k]))
                    ]
                    first_direct.append(seq[0])
                    last_direct.append(seq[-1])

                for ki in range(K):
                    x_t = xpool.tile([C, A, B, PSH], f32r, tag="xt")
                    nc.sync.dma_start(x_t[:], x_dram[:, ki])
                    for j, r in enumerate(rs):
                        has_bins = len(plan[r][1]) > 0
                        for i, (a, kk, aa) in enumerate(direct_by_k[j][ki]):
                            nc.tensor.matmul(
                                psum[:, j, :, :],
                                w_t[:, kk, aa, :],
                                x_t[:, a, :, :],
                                start=((ki, i) == first_direct[j]),
                                stop=(
                                    not has_bins and (ki, i) == last_direct[j]
                                ),
                            )
                        # DVE: fold this k-tile's sources into bins
                        for bi, (kkaa, srcs) in enumerate(plan[r][1]):
                            for (k, a) in srcs:
                                if k != ki:
                                    continue
                                if (k, a) == srcs[0]:
                                    nc.vector.tensor_copy(
                                        bins_t[:, j, bi, :, :], x_t[:, a, :, :]
                                    )
                                else:
                                    nc.vector.tensor_add(
                                        bins_t[:, j, bi, :, :],
                                        bins_t[:, j, bi, :, :],
                                        x_t[:, a, :, :],
                                    )

                # pass tail: one matmul per offloaded bin
                for j, r in enumerate(rs):
                    nbins = len(plan[r][1])
                    for bi, ((kk, aa), srcs) in enumerate(plan[r][1]):
                        nc.tensor.matmul(
                            psum[:, j, :, :],
                            w_t[:, kk, aa, :],
                            bins_t[:, j, bi, :, :],
                            start=False,
                            stop=(bi == nbins - 1),
                        )

                for j in range(6):
                    nc.vector.tensor_copy(
                        out_s[:, :, :, 6 * h + j], psum[:, j, :, :]
                    )

            for b in range(B):
                nc.sync.dma_start(out_dram[b], out_s[:, b, :, :])

    nc.compile()
    return nc


def _prep_inputs(x, W):
    # W (d, c, kk, aa) -> (c, kk, aa, d): stationary [c, d] slices contiguous
    w_perm = np.ascontiguousarray(W.transpose(1, 2, 3, 0))
    in_maps = []
    for i in range(NCORES):
        # (B, C, K, PSH, A) -> (C, K, A, B, PSH)
        xs = np.ascontiguousarray(
            x[:, :, :, i * PSH : (i + 1) * PSH, :].transpose(1, 2, 4, 0, 3)
        )
        in_maps.append({"x": xs, "w": w_perm})
    return in_maps


def _build_for_bench_v1(inputs, iters):
    x = np.asarray(inputs["x"], dtype=np.float32)
    W = np.asarray(inputs["W"], dtype=np.float32)
    kidx = np.asarray(inputs["kidx"], dtype=np.int32)
    aidx = np.asarray(inputs["aidx"], dtype=np.int32)
    nc = _build(kidx, aidx, iters=iters)
    return nc, _prep_inputs(x, W)


def _kernel_v1(x, W, kidx, aidx):
    from concourse.bass_utils import run_bass_kernel_spmd

    x = np.asarray(x, dtype=np.float32)
    W = np.asarray(W, dtype=np.float32)
    kidx = np.asarray(kidx, dtype=np.int32)
    aidx = np.asarray(aidx, dtype=np.int32)

    key = (kidx.tobytes(), aidx.tobytes())
    nc = _cache.get(key)
    if nc is None:
        nc = _build(kidx, aidx)
        _cache[key] = nc

    in_maps = _prep_inputs(x, W)

    res = run_bass_kernel_spmd(nc, in_maps, list(range(NCORES)))
    out = np.concatenate([res.results[i]["out"] for i in range(NCORES)], axis=2)
    return out


# ===== fast path =====

NBP = 2
BCOL = 2  # b's per pair
MM_NS = 107.0  # 256-col fp32r matmul
DVE_OP_NS = 3280.0  # [128, 3072] fp32 tensor op on DVE
POOL_OP_NS = 2660.0  # same on Pool
DMA_SLICE_NS = 4365.0
ACT_OP_NS = 2900.0  # [128, 3072] f32 activation copy
MARGIN_NS = 3000.0
CONV_N = 6  # adjacency terms converted to direct-PE matmuls

def _analyze(kidx3, aidx3):
    """Check icosahedral structure; return per-r info or None."""
    if kidx3.shape != (K, A, R) or aidx3.shape != (K, A, R):
        return None
    if not (kidx3 == kidx3[:, :1, :]).all():
        return None
    if not (aidx3 == aidx3[:1, :, :]).all():
        return None
    kidx = kidx3[:, 0, :]  # (K, R)
    aidx = aidx3[0]  # (A, R)
    info = []
    for r in range(R):
        perm = aidx[:, r]
        if sorted(perm.tolist()) != list(range(A)):
            return None
        ainv = np.argsort(perm)  # ainv[aa] = a such that perm[a] == aa
        sets = {v: [k for k in range(K) if kidx[k, r] == v] for v in range(5)}
        if [len(sets[v]) for v in range(5)] != [1, 5, 5, 1, 1]:
            return None
        if sets[4] != [K - 1]:
            return None
        if sorted(sets[0] + sets[1] + sets[2] + sets[3]) != list(range(K - 1)):
            return None
        info.append(
            dict(ainv=ainv, k0=sets[0][0], adj=sets[1], k3=sets[3][0])
        )
    return info


def _schedule(info):
    """Exact per-b-pair event model: walk the emission loop with
    arrival gates and in-order engine queues to predict each A_r's
    completion; phases are placed at those times."""
    verts = list(range(K - 1))
    dma_order = [K - 1] + verts
    arrival = {k: DMA_SLICE_NS * (i + 1) for i, k in enumerate(dma_order)}
    last_adj = {r: max(arrival[k] for k in info[r]["adj"]) for r in range(R)}
    center = {info[r]["k0"]: r for r in range(R)}
    partner = {r: center[info[r]["k3"]] for r in range(R)}
    pairs, seen = [], set()
    for r in range(R):
        if r not in seen:
            pairs.append((r, partner[r]))
            seen.update(pairs[-1])
    # convert the later-completing member of CONV_N pairs; its partner
    # keeps a materialized A so the conv term can be derived:
    #   W1'A_r + W2 T = W1 T - W1' x_c - W1' x_o - W1' A_partner
    pairs.sort(key=lambda p: -max(last_adj[p[0]], last_adj[p[1]]))
    conv_set = [max(p, key=lambda r: last_adj[r]) for p in pairs[:CONV_N]]
    a_rs = [r for r in range(R) if r not in conv_set]
    # engine split (greedy by predicted total load)
    a_engine = {}
    loads = {"dve": 6 * DVE_OP_NS, "pool": 4 * POOL_OP_NS}  # T chain parts
    opns = {"dve": DVE_OP_NS, "pool": POOL_OP_NS}
    for r in sorted(a_rs, key=lambda r: last_adj[r]):
        e = min(loads, key=lambda e: loads[e] + 4 * opns[e])
        a_engine[r] = e
        loads[e] += 4 * opns[e]
    # exact event sim of the emission loop
    free = {"dve": 0.0, "pool": 0.0, "act": 0.0}
    done = {}
    t_done = 0.0
    for vi, k in enumerate(verts):
        if vi == 1:
            free["pool"] = max(free["pool"], arrival[k]) + POOL_OP_NS
            t_done = free["pool"]
        elif 2 <= vi < 6:
            free["pool"] = max(free["pool"], arrival[k], t_done) + POOL_OP_NS
            t_done = free["pool"]
        elif vi >= 6:
            free["dve"] = max(free["dve"], arrival[k], t_done) + DVE_OP_NS
            t_done = free["dve"]
        for r in a_rs:
            if k not in info[r]["adj"]:
                continue
            e = a_engine[r]
            if r not in done:
                free["act"] = max(free["act"], arrival[k]) + ACT_OP_NS
                done[r] = free["act"]
            else:
                free[e] = max(free[e], arrival[k], done[r]) + opns[e]
                done[r] = free[e]
    a_ready = {r: done[r] + MARGIN_NS for r in a_rs}
    t_ready = t_done + MARGIN_NS
    return dict(
        dma_order=dma_order,
        arrival=arrival,
        conv_set=conv_set,
        a_rs=a_rs,
        a_engine=a_engine,
        a_ready=a_ready,
        t_ready=t_ready,
        verts=verts,
        partner=partner,
    )


def _build_v5(kidx3: np.ndarray, aidx3: np.ndarray, iters: int = 1):
    import concourse.bass as bass  # noqa: F401
    import concourse.tile as tile
    from concourse import bacc, mybir

    info = _analyze(kidx3, aidx3)
    assert info is not None
    sch = _schedule(info)

    f32 = mybir.dt.float32
    f32r = mybir.dt.float32r
    bf16 = mybir.dt.bfloat16

    nc = bacc.Bacc(
        "TRN2", target_bir_lowering=False, debug=False, num_devices=NCORES
    )
    x_dram = nc.dram_tensor(
        "x", [NBP, K, C, A, BCOL, PSH], f32r, kind="ExternalInput"
    ).ap()
    w_dram = nc.dram_tensor(
        "w", [C, 9 * A + 1, D], f32r, kind="ExternalInput"
    ).ap()
    out_dram = nc.dram_tensor(
        "out", [R, D, NBP, BCOL, PSH], bf16, kind="ExternalOutput"
    ).ap()

    # direct (g0/g3) jobs per vertex slice
    ZROW = 5 * A
    W1_BASE = 5 * A + 1
    NEG_BASE = W1_BASE + A
    W01_BASE = NEG_BASE + A  # W0 - W1 (conv center jobs)
    W31_BASE = W01_BASE + A  # W3 - W1 (conv opposite jobs)
    jobs = {k: [] for k in range(K - 1)}
    for r in range(R):
        if r in sch["conv_set"]:
            # combined stationaries fold the negated-W1' slice terms
            # into the center/opposite jobs: conv r costs 60 matmuls
            jobs[info[r]["k0"]].append((r, W01_BASE))
            jobs[info[r]["k3"]].append((r, W31_BASE))
        else:
            jobs[info[r]["k0"]].append((r, 0 * A))
            jobs[info[r]["k3"]].append((r, 3 * A))

    with tile.TileContext(nc) as tc, ExitStack() as ctx:
        wpool = ctx.enter_context(tc.tile_pool(name="wpool", bufs=1))
        xpool = ctx.enter_context(tc.tile_pool(name="xpool", bufs=6))
        tpool = ctx.enter_context(tc.tile_pool(name="tpool", bufs=1))
        apool = ctx.enter_context(tc.tile_pool(name="apool", bufs=5))
        spool = ctx.enter_context(tc.tile_pool(name="spool", bufs=2))
        ppool = ctx.enter_context(
            tc.tile_pool(name="ppool", bufs=1, space="PSUM")
        )

        w_t = wpool.tile([C, 9 * A + 1, D], f32r)
        nc.sync.dma_start(w_t[:], w_dram[:])

        for _ in range(iters):
            for bp in range(NBP):
                xk = {}
                for k in sch["dma_order"]:
                    xk[k] = xpool.tile(
                        [C, A, BCOL, PSH], f32r, tag="xs", name=f"x{bp}_{k}"
                    )
                    nc.sync.dma_start(xk[k][:], x_dram[bp, k])

                # --- vector ops, strictly in slice-arrival order:
                # T chain (Pool first 6 verts, DVE rest) and incremental
                # A_r accumulation on the assigned engine
                t_t = tpool.tile([C, A, BCOL, PSH], f32r, tag="T")
                a_tiles = {}
                a_seen = {r: 0 for r in sch["a_rs"]}
                verts = sch["verts"]
                for vi, k in enumerate(verts):
                    if vi == 1:
                        nc.gpsimd.tensor_add(
                            t_t[:], xk[verts[0]][:], xk[verts[1]][:]
                        )
                    elif vi >= 2 and vi < 6:
                        nc.gpsimd.tensor_add(t_t[:], t_t[:], xk[k][:])
                    elif vi >= 6:
                        nc.vector.tensor_add(t_t[:], t_t[:], xk[k][:])
                    for r in sch["a_rs"]:
                        if k not in info[r]["adj"]:
                            continue
                        eng = (
                            nc.vector
                            if sch["a_engine"][r] == "dve"
                            else nc.gpsimd
                        )
                        a_seen[r] += 1
                        if a_seen[r] == 1:
                            # init by Act copy so this slice's read happens
                            # now (keeps the x slot pool recycling)
                            at = apool.tile(
                                [C, A, BCOL, PSH],
                                f32r,
                                tag="a",
                                name=f"a{bp}_{r}",
                            )
                            a_tiles[r] = at
                            nc.scalar.copy(at[:], xk[k][:])
                        else:
                            eng.tensor_add(
                                a_tiles[r][:], a_tiles[r][:], xk[k][:]
                            )

                # --- PE stream
                ps = ppool.tile([D, R, BCOL, PSH], f32, tag="ps")
                r_rem = [5 * A] * R  # all r: 5 groups x 12 aa
                bank_rem = [2 * 5 * A] * (R // 2)

                def mm(r, row_base, aa, moving):
                    nc.tensor.matmul(
                        ps[:, r],
                        w_t[:, row_base + aa],
                        moving,
                        start=False,
                        stop=bank_rem[r // 2] == 1,
                    )
                    bank_rem[r // 2] -= 1
                    r_rem[r] -= 1
                    if bank_rem[r // 2] == 0:
                        # bank group closed: both r's are final, drain them
                        for rr in (2 * (r // 2), 2 * (r // 2) + 1):
                            st = spool.tile(
                                [D, BCOL, PSH],
                                bf16,
                                tag="st",
                                name=f"s{bp}{rr}",
                            )
                            nc.scalar.copy(st[:], ps[:, rr])
                            nc.scalar.dma_start(out_dram[rr, :, bp], st[:])

                for j in range(R // 2):
                    nc.tensor.matmul(
                        ps[:, 2 * j : 2 * j + 2],
                        w_t[:, 5 * A],
                        w_t[:, 4 * j : 4 * j + 4],
                        start=True,
                        stop=False,
                    )

                t = 0.0
                pend = sorted(sch["a_rs"], key=lambda r: sch["a_ready"][r])
                tpend = list(range(R))

                def phase(r, base, tile):
                    nonlocal t
                    ainv = info[r]["ainv"]
                    for aa in range(A):
                        mm(r, base, aa, tile[:, ainv[aa]])
                    t += A * MM_NS

                cpart = {
                    sch["partner"][cr]: cr for cr in sch["conv_set"]
                }  # a_r -> conv partner whose negA-phase follows a_r's

                def try_phases():
                    nonlocal t
                    while pend and sch["a_ready"][pend[0]] <= t:
                        r = pend.pop(0)
                        phase(r, 1 * A, a_tiles[r])
                        if r in cpart:
                            # conv partner: -W1' x A_r term
                            phase(cpart[r], NEG_BASE, a_tiles[r])
                    while tpend and sch["t_ready"] <= t:
                        r = tpend.pop(0)
                        base = W1_BASE if r in sch["conv_set"] else 2 * A
                        phase(r, base, t_t)

                for r in range(R):
                    phase(r, 4 * A, xk[K - 1])
                    try_phases()
                for k in sch["dma_order"][1:]:
                    for r, base in jobs[k]:
                        phase(r, base, xk[k])
                        try_phases()
                for r in pend:
                    phase(r, 1 * A, a_tiles[r])
                    if r in cpart:
                        phase(cpart[r], NEG_BASE, a_tiles[r])
                for r in tpend:
                    base = W1_BASE if r in sch["conv_set"] else 2 * A
                    phase(r, base, t_t)

    nc.compile()
    return nc


def _prep_v5(x, W):
    W = W.astype(np.float32)
    W2 = W[:, :, 2, :]
    gs = np.stack(
        [
            W[:, :, 0, :] - W2,
            W[:, :, 1, :] - W2,
            W2,
            W[:, :, 3, :] - W2,
            W[:, :, 4, :],
        ],
        axis=0,
    )  # (5, D, C, A)
    w_perm = np.ascontiguousarray(
        gs.transpose(2, 0, 3, 1).reshape(C, 5 * A, D)
    ).astype(np.float32)
    W1p = np.ascontiguousarray(
        W[:, :, 1, :].transpose(1, 2, 0)
    ).astype(np.float32)  # (C, A, D)
    negW1q = np.ascontiguousarray(
        (W2 - W[:, :, 1, :]).transpose(1, 2, 0)
    ).astype(np.float32)  # -(W1 - W2)
    W01 = np.ascontiguousarray(
        (W[:, :, 0, :] - W[:, :, 1, :]).transpose(1, 2, 0)
    ).astype(np.float32)
    W31 = np.ascontiguousarray(
        (W[:, :, 3, :] - W[:, :, 1, :]).transpose(1, 2, 0)
    ).astype(np.float32)
    w_perm = np.concatenate(
        [w_perm, np.zeros((C, 1, D), np.float32), W1p, negW1q, W01, W31],
        axis=1
    )
    in_maps = []
    for i in range(NCORES):
        xs = x[:, :, :, i * PSH : (i + 1) * PSH, :]  # (B, C, K, PSH, A)
        t = xs.transpose(2, 1, 4, 0, 3)  # (K, C, A, B, PSH)
        t = t.reshape(K, C, A, NBP, BCOL, PSH).transpose(3, 0, 1, 2, 4, 5)
        in_maps.append(
            {"x": np.ascontiguousarray(t).astype(np.float32), "w": w_perm}
        )
    return in_maps


def _assemble_v5(res):
    outs = []
    for i in range(NCORES):
        arr = np.asarray(res.results[i]["out"]).astype(np.float32)
        # (R, D, NBP, BCOL, PSH) -> (B, D, PSH, R)
        o = arr.transpose(2, 3, 1, 4, 0).reshape(B, D, PSH, R)
        outs.append(o)
    return np.concatenate(outs, axis=2)


def _get_nc_v5(kidx, aidx, iters=1):
    key = ("v5", kidx.tobytes(), aidx.tobytes(), iters)
    nc = _cache.get(key)
    if nc is None:
        nc = _build_v5(kidx, aidx, iters=iters)
        _cache[key] = nc
    return nc


def build_for_bench(inputs, iters):
    x = np.asarray(inputs["x"], dtype=np.float32)
    W = np.asarray(inputs["W"], dtype=np.float32)
    kidx = np.asarray(inputs["kidx"], dtype=np.int32)
    aidx = np.asarray(inputs["aidx"], dtype=np.int32)
    nc = _get_nc_v5(kidx, aidx, iters=iters)
    return nc, _prep_v5(x, W)


def kernel(x, W, kidx, aidx):
    from concourse.bass_utils import run_bass_kernel_spmd

    x = np.asarray(x, dtype=np.float32)
    W = np.asarray(W, dtype=np.float32)
    kidx = np.asarray(kidx, dtype=np.int32)
    aidx = np.asarray(aidx, dtype=np.int32)

    if _analyze(kidx, aidx) is None:
        return _kernel_v1(x, W, kidx, aidx)

    nc = _get_nc_v5(kidx, aidx)
    in_maps = _prep_v5(x, W)
    res = run_bass_kernel_spmd(nc, in_maps, list(range(NCORES)))
    return _assemble_v5(res)

